# revision 21
# baseline (speedup 1.0000x reference)
"""BasicMoEBlock kernel for Trainium2 (Bass/Tile), data-parallel over batch on 8 cores.

Computation per sample (matches the reference):
    rw1 = avgpool_experts(sigmoid(mean_hw(x) @ r1_W.T + r1_b))
    out = relu(bn1(conv3x3(x, rw1 @ e1_w)))
    rw2 = avgpool_experts(sigmoid(mean_hw(out) @ r2_W.T + r2_b))
    out = relu(bn2(conv3x3(out, rw2 @ e2_w)) + x)

Mapping:
  - conv3x3 = 18 accumulating PE matmuls (2 ci-chunks x 9 shifts) over a
    zero-padded 34x34 image held in SBUF (bf16), fp32 PSUM accumulation.
    ci-outer loop order so a conv can start as soon as the first ci-half of
    its combined weights exists (~36 matmuls of cover for the second half).
  - x arrives from the host already padded + bf16 (pure layout/dtype
    marshalling): no on-device casts, and channel pooling is a 4x-rate DVE
    tensor_scalar with accum_out (padding is zero, so the flat sum is exact).
  - per-sample expert-weight combination: e0/e1 mults + the 3 adds on DVE,
    e2/e3 mults on ACT. Sample-0 layer-1 combines in co-half quarters so
    the first conv starts as soon as the first MB of weights lands.
  - all DMA on the single sync HWDGE ring, ordered by need-time: x0, the
    sample-0 weight quarters, x1..3, the rest of ew1, then ew2.
"""

import numpy as np
import ml_dtypes

import concourse.bass as bass
import concourse.tile as tile
from concourse import mybir

F32 = mybir.dt.float32
BF16 = mybir.dt.bfloat16
BF16_NP = ml_dtypes.bfloat16

N_CORES = 8
B_LOC = 4          # samples per core
P = 128            # partitions
CI2 = 2            # input-channel chunks (256 = 2*128)
CO2 = 2            # output-channel chunks
C = 256
HW = 1024          # 32*32
PADW = 34
PADHW = PADW * PADW
E = 4              # experts
NSH = 9            # 3x3 shifts
EPS = 1e-5
AF = mybir.ActivationFunctionType
OP = mybir.AluOpType
QW = NSH * P       # weight elements per (ci, co) quarter = 1152


# ---------------------------------------------------------------- kernel build

def _declare_io(nc):
    d = {}

    def din(name, shape, dtype):
        d[name] = nc.dram_tensor(name, shape, dtype, kind="ExternalInput").ap()

    din("xp", [B_LOC, CI2, P, PADHW], BF16)   # pre-padded bf16 x
    din("ew1", [P, E, CI2, CO2, QW], BF16)
    din("ew2", [P, E, CI2, CO2, QW], BF16)
    din("rwt", [P, 2, CI2, C], BF16)    # r{1,2}_W.T, [cin_in, layer, cin_chunk, interm]
    # fp32 blob: rb1[2] rb2[2] inv1[2] shift1[2] inv2[2] shift2[2] mask4[4]
    din("fblob", [P, 16], F32)
    d["out"] = nc.dram_tensor("out", [B_LOC, C, HW], F32, kind="ExternalOutput").ap()
    return d


def _emit(tc, d):
    nc = tc.nc

    with (
        tc.tile_pool(name="const", bufs=1) as const,
        tc.tile_pool(name="wcombp", bufs=3) as wcombp,
        tc.tile_pool(name="wtmp", bufs=6) as wtmpp,
        tc.tile_pool(name="resp", bufs=3) as resp,
        tc.tile_pool(name="rsb", bufs=4) as rsb,
        tc.tile_pool(name="rps", bufs=2, space="PSUM") as rps,
        tc.tile_pool(name="cps", bufs=3, space="PSUM") as cps,
    ):
        # ---- persistent state
        ew_sb = [const.tile([P, E, CI2, CO2, QW], BF16, tag=f"ew{l}", name=f"ew{l}") for l in (0, 1)]
        rwt_all = const.tile([P, 2, CI2, C], BF16, tag="rwtall")
        fblob = const.tile([P, 16], F32, tag="fblob")
        rwt_sb = [rwt_all[:, l] for l in (0, 1)]
        pool_bf = [const.tile([P, B_LOC, CI2], BF16, tag=f"poolbf{l}", name=f"poolbf{l}") for l in (0, 1)]
        rb_sb = [fblob[:, 0:2], fblob[:, 2:4]]
        inv_sb = [fblob[:, 4:6], fblob[:, 8:10]]
        shift_sb = [fblob[:, 6:8], fblob[:, 10:12]]
        mask_sb = fblob[:, 12:16]
        ones_pp = const.tile([P, P], BF16, tag="onespp")
        xpad = const.tile([P, B_LOC, CI2, PADHW], BF16, tag="xpad")
        o1pad = const.tile([P, B_LOC, CI2, PADHW], BF16, tag="o1pad")
        pool_acc = [const.tile([P, B_LOC, CI2], F32, tag=f"pool{l}", name=f"pool{l}") for l in (0, 1)]
        pool_pc = const.tile([P, CI2, 2], F32, tag="poolpc")   # b0 piece accums
        rwbc = [const.tile([P, B_LOC, E], F32, tag=f"rwbc{l}", name=f"rwbc{l}") for l in (0, 1)]
        pscr = const.tile([P, PADHW], BF16, tag="pscr")    # pooling scratch (DVE)
        pscr2 = const.tile([P, PADHW], BF16, tag="pscr2")  # pooling scratch (ACT)

        # ---- input DMA: one ring (sync HWDGE), strictly ordered by when the
        # pipeline needs each tensor. Issue cost is ~0.65us per DMA; sample-0's
        # path needs fblob+x0+rwt+ew1[ci0] first. Expert weights are DMA'd in
        # e-pairs per (ci, co) quarter: (e0,e1) feed DVE, (e2,e3) feed ACT.
        nc.sync.dma_start(out=fblob, in_=d["fblob"])
        # x0 split into half-chunks so pooling starts on the first piece
        for c in range(CI2):
            for h in range(2):
                sl = slice(h * (PADHW // 2), (h + 1) * (PADHW // 2))
                nc.sync.dma_start(out=xpad[:, 0, c, sl], in_=d["xp"][0, c][:, sl])
        nc.sync.dma_start(out=rwt_all, in_=d["rwt"])

        def ew_pair(l, ep, ci, co):
            nc.sync.dma_start(
                out=ew_sb[l][:, 2 * ep : 2 * ep + 2, ci, co],
                in_=d[f"ew{l + 1}"][:, 2 * ep : 2 * ep + 2, ci, co],
            )

        ew_pair(0, 0, 0, 0)
        ew_pair(0, 1, 0, 0)
        for c in range(CI2):
            nc.sync.dma_start(out=xpad[:, 1, c], in_=d["xp"][1, c])
        ew_pair(0, 0, 0, 1)
        ew_pair(0, 1, 0, 1)
        for co in range(CO2):
            ew_pair(0, 0, 1, co)
            ew_pair(0, 1, 1, co)
        for b in (2, 3):
            for c in range(CI2):
                nc.sync.dma_start(out=xpad[:, b, c], in_=d["xp"][b, c])
        for e in range(E):
            nc.sync.dma_start(out=ew_sb[1][:, e], in_=d["ew2"][:, e])

        # layer-2 pad borders (layer-1 borders arrive zeroed from the host)
        vo = o1pad.rearrange("p b c (r q) -> p b c r q", r=PADW)
        nc.vector.memset(vo[:, :, :, 0:PADW:33, :], 0.0)
        nc.vector.memset(vo[:, :, :, 1:33, 0:PADW:33], 0.0)
        nc.vector.memset(ones_pp, 1.0)

        # warm the ACT function-table with the sigmoid set before the first
        # real sigmoid lands mid-critical-path (table switch costs ~1.3us)
        warm = rsb.tile([P, 1], F32, tag="warm")
        warm_inst = nc.scalar.activation(
            out=warm, in_=fblob[:, 0:1], func=AF.Sigmoid, scale=1.0
        )
        first_sig = [None]

        def poolx(b):
            # layer-1 channel pooling: free-dim sum via tensor_scalar
            # accum_out (borders are zero; sigmoid's scale folds the 1/HW).
            # b0 pools per half-chunk piece (tracking its split DMA, accum
            # slots joined by routing's first op); b1-3 pool whole chunks.
            if b == 0:
                hw2 = PADHW // 2
                for c in range(CI2):
                    for h in range(2):
                        nc.vector.tensor_scalar(
                            out=pscr[:, :hw2],
                            in0=xpad[:, 0, c, h * hw2 : (h + 1) * hw2],
                            scalar1=1.0, scalar2=0.0,
                            op0=OP.mult, op1=OP.add,
                            accum_out=pool_pc[:, c, h : h + 1],
                        )
                return
            for c in range(CI2):
                if b >= 2:
                    # ACT has slack in the layer-1 window; keep DVE for combines
                    nc.scalar.activation(
                        out=pscr2, in_=xpad[:, b, c], func=AF.Copy, scale=1.0,
                        accum_out=pool_acc[0][:, b, c : c + 1],
                    )
                else:
                    nc.vector.tensor_scalar(
                        out=pscr, in0=xpad[:, b, c], scalar1=1.0, scalar2=0.0,
                        op0=OP.mult, op1=OP.add,
                        accum_out=pool_acc[0][:, b, c : c + 1],
                    )

        def routing(b0, n, l):
            """pool_acc[l][:, b0:b0+n] -> rwbc[l][:, b0:b0+n] for n samples."""
            if l == 0 and b0 == 0 and n == 1:
                # join sample-0's piece accumulators (cast to bf16 en route)
                nc.vector.tensor_add(
                    pool_bf[0][:, 0], pool_pc[:, :, 0], pool_pc[:, :, 1]
                )
            else:
                nc.vector.tensor_copy(
                    pool_bf[l][:, b0 : b0 + n], pool_acc[l][:, b0 : b0 + n]
                )
            rt_ps = rps.tile([P, CI2, n], F32, tag="rpsA", name="rtps")
            for ic in range(2):
                for cc in range(2):
                    nc.tensor.matmul(
                        rt_ps[:, ic],
                        rwt_sb[l][:, cc, ic * P : (ic + 1) * P],
                        pool_bf[l][:, b0 : b0 + n, cc],
                        start=(cc == 0),
                        stop=(cc == 1),
                    )
            rt2 = rsb.tile([P, CI2, n], F32, tag="rt2", name="rt2")
            for ic in range(2):
                si = nc.scalar.activation(
                    out=rt2[:, ic],
                    in_=rt_ps[:, ic],
                    func=AF.Sigmoid,
                    bias=rb_sb[l][:, ic : ic + 1],
                    scale=1.0 / HW,
                )
                if first_sig[0] is None:
                    first_sig[0] = si
                    tile.add_dep_helper(
                        warm_inst.ins, si.ins, sync=False,
                        reason="act table preload",
                    )
            # masked[p, bb, e] = rt2[p, e>>1, bb] * mask[p, e] (bf16)
            rt_g = bass.AP(
                tensor=rt2.tensor,
                offset=rt2.offset,
                ap=[rt2.ap[0], [1, n], [n, 2], [0, 2]],
            )
            msk_g = bass.AP(
                tensor=mask_sb.tensor,
                offset=mask_sb.offset,
                ap=[mask_sb.ap[0], [0, n], [2, 2], [1, 2]],
            )
            masked = rsb.tile([P, n, E], BF16, tag="masked", name="masked")
            nc.vector.tensor_mul(
                masked.rearrange("p b (h i) -> p b h i", h=2), rt_g, msk_g
            )
            # all-ones stationary: one matmul both sums over partitions and
            # broadcasts the result to every partition
            rwbc_ps = rps.tile([P, n * E], F32, tag="rpsA", name="rwbcps")
            nc.tensor.matmul(
                rwbc_ps, ones_pp, masked.rearrange("p b e -> p (b e)"),
                start=True, stop=True,
            )
            nc.vector.tensor_copy(
                rwbc[l][:, b0 : b0 + n].rearrange("p b e -> p (b e)"), rwbc_ps
            )

        def wcomb_mac(b, l, quarters=False, dve_pieces=0):
            """combined per-sample conv weights: sum_e rw[b,e] * ew[e]  (bf16).
            e0/e1 mults + the 3 adds on DVE (tensor_scalar 4x, tensor_tensor
            2x); e2/e3 mults ride ACT scale-copies. quarters=True emits per
            (ci, co) quarter (startup path, tracks the staged weight DMAs);
            dve_pieces: the first N pieces stay fully on DVE (sample 0's
            first quarter: no cross-engine joins on the critical path)."""
            w = wcombp.tile([P, CI2, CO2, QW], BF16, tag="wcomb")
            sc = [rwbc[l][:, b, e : e + 1] for e in range(E)]
            pieces = (
                [(ci, co) for ci in range(CI2) for co in range(CO2)]
                if quarters else [(ci, None) for ci in range(CI2)]
            )
            for pi, (ci, co) in enumerate(pieces):
                dve_only = pi < dve_pieces
                if co is None:
                    wv = w[:, ci].rearrange("p c q -> p (c q)")
                    src = [ew_sb[l][:, e, ci].rearrange("p c q -> p (c q)")
                           for e in range(E)]
                else:
                    wv = w[:, ci, co]
                    src = [ew_sb[l][:, e, ci, co] for e in range(E)]

                def tmp():
                    return wtmpp.tile(
                        [P, CI2 * QW], BF16, tag="wtmp", name="mt"
                    )[:, : wv.free_size()]

                nc.vector.tensor_scalar(
                    out=wv, in0=src[0], scalar1=sc[0], scalar2=None, op0=OP.mult
                )
                if dve_only:
                    for e in (1, 2, 3):
                        m = tmp()
                        nc.vector.tensor_scalar(
                            out=m, in0=src[e], scalar1=sc[e], scalar2=None,
                            op0=OP.mult,
                        )
                        nc.vector.tensor_add(wv, wv, m)
                else:
                    m2, m3, m1 = tmp(), tmp(), tmp()
                    nc.scalar.activation(out=m2, in_=src[2], func=AF.Copy, scale=sc[2])
                    nc.scalar.activation(out=m3, in_=src[3], func=AF.Copy, scale=sc[3])
                    nc.vector.tensor_scalar(
                        out=m1, in0=src[1], scalar1=sc[1], scalar2=None, op0=OP.mult
                    )
                    nc.vector.tensor_add(wv, wv, m1)
                    nc.vector.tensor_add(wv, wv, m2)
                    nc.vector.tensor_add(wv, wv, m3)
            return w

        def conv(b, w, srcpad, hh_outer=False):
            """3x3 same conv: 18 accumulating matmuls per (co, h-half) into
            two [P, 1024] fp32 psum tiles (co chunks). Default ci-outer order
            so only w[:, 0] gates the start; hh_outer (last conv) finishes
            each h-half's group early so the epilogue overlaps the tail."""
            psums = [cps.tile([P, HW], F32, tag="convps", name=f"ps{co}") for co in range(2)]
            if hh_outer:
                order = [(co, hh, ci, s) for co in range(2) for hh in range(2)
                         for ci in range(2) for s in range(NSH)]
            else:
                order = [(co, hh, ci, s) for ci in range(2) for co in range(2)
                         for s in range(NSH) for hh in range(2)]
            for co, hh, ci, s in order:
                ky, kx = divmod(s, 3)
                src34 = srcpad[:, b, ci].rearrange("p (r q) -> p r q", r=PADW)
                rhs = src34[:, ky + hh * 16 : ky + hh * 16 + 16, kx : kx + 32]
                nc.tensor.matmul(
                    psums[co][:, hh * 512 : (hh + 1) * 512],
                    w[:, ci, co, s * P : (s + 1) * P],
                    rhs,
                    start=(ci == 0 and s == 0),
                    stop=(ci == 1 and s == NSH - 1),
                )
            return psums

        def bn1_relu(b, psums):
            for co in range(2):
                dst = o1pad[:, b, co].rearrange("p (r q) -> p r q", r=PADW)[:, 1:33, 1:33]
                nc.scalar.activation(
                    out=dst,
                    in_=psums[co].rearrange("p (r q) -> p r q", r=32),
                    func=AF.Relu,
                    bias=shift_sb[0][:, co : co + 1],
                    scale=inv_sb[0][:, co : co + 1],
                    accum_out=pool_acc[1][:, b, co : co + 1],
                )

        def bn2_res(b, psums, split=False):
            halves = [None] if not split else list(range(2))
            for co in range(2):
                res = resp.tile([P, HW], F32, tag="res")
                for hh in halves:
                    sl = slice(None) if hh is None else \
                        slice(hh * 512, (hh + 1) * 512)
                    rows = 32 if hh is None else 16
                    r0 = 0 if hh is None else hh * 16
                    resv = res[:, sl].rearrange("p (r q) -> p r q", r=rows)
                    xv = xpad[:, b, co].rearrange("p (r q) -> p r q", r=PADW)[
                        :, 1 + r0 : 1 + r0 + rows, 1:33]
                    psv = psums[co][:, sl].rearrange("p (r q) -> p r q", r=rows)
                    # res = psum*inv2 + x ; res = max(res + shift2, 0)
                    nc.vector.scalar_tensor_tensor(
                        out=resv, in0=psv, scalar=inv_sb[1][:, co : co + 1], in1=xv,
                        op0=OP.mult, op1=OP.add,
                    )
                    if not split:
                        nc.scalar.activation(
                            out=res[:, sl], in_=res[:, sl], func=AF.Relu,
                            bias=shift_sb[1][:, co : co + 1], scale=1.0,
                        )
                        nc.sync.dma_start(
                            out=d["out"][b, co * P : (co + 1) * P, sl],
                            in_=res[:, sl],
                        )
                        continue
                    # drain path: shift+relu on DVE (no cross-engine hop) and
                    # finer pieces at the very end so the last DMA leaves early
                    npc = 2 if (co == 1 and hh == 1) else 1
                    for pc in range(npc):
                        w = 512 // npc
                        psl = slice(hh * 512 + pc * w, hh * 512 + (pc + 1) * w)
                        nc.vector.tensor_scalar(
                            out=res[:, psl], in0=res[:, psl],
                            scalar1=shift_sb[1][:, co : co + 1], scalar2=0.0,
                            op0=OP.add, op1=OP.max,
                        )
                        nc.sync.dma_start(
                            out=d["out"][b, co * P : (co + 1) * P, psl],
                            in_=res[:, psl],
                        )

        # ---- main pipeline
        # layer-1 routing/combines, sample 0's path shortest (quarters)
        w1 = {}
        poolx(0)
        routing(0, 1, 0)
        w1[0] = wcomb_mac(0, 0, quarters=True, dve_pieces=1)
        poolx(1)
        routing(1, 1, 0)
        w1[1] = wcomb_mac(1, 0)
        poolx(2)
        poolx(3)
        routing(2, 2, 0)
        w1[2] = wcomb_mac(2, 0)
        w1[3] = wcomb_mac(3, 0)

        w2 = {}
        for b in range(B_LOC):
            ps = conv(b, w1[b], xpad)
            bn1_relu(b, ps)
            if b == 1:
                routing(0, 2, 1)
                w2[0] = wcomb_mac(0, 1)
                w2[1] = wcomb_mac(1, 1)
            elif b == 3:
                routing(2, 2, 1)
                w2[2] = wcomb_mac(2, 1)
                w2[3] = wcomb_mac(3, 1)
        for b in range(B_LOC):
            last = b == B_LOC - 1
            ps = conv(b, w2[b], o1pad, hh_outer=last)
            bn2_res(b, ps, split=last)


_NC_CACHE = {}


def _build_nc():
    if "nc" not in _NC_CACHE:
        import concourse.bacc as bacc

        # Bacc (not raw Bass): its compile() runs split_sync_waits, which
        # legalizes multi-wait instructions for TRN2's 1-wait-per-inst ISA.
        nc = bacc.Bacc("TRN2", target_bir_lowering=False)
        d = _declare_io(nc)
        with tile.TileContext(nc) as tc:
            _emit(tc, d)
        nc.compile()
        _NC_CACHE["nc"] = nc
    return _NC_CACHE["nc"]


# ---------------------------------------------------------------- host prep

def _prep_ew(e_w):
    # [4, 589824] -> [ci_in(128), e, ci_chunk, co_chunk, (ky kx co_in)]  bf16
    w = np.asarray(e_w, np.float32).reshape(E, CO2, P, CI2, P, 3, 3)
    # -> ci_in, e, ci_chunk, co_chunk, ky, kx, co_in
    w = w.transpose(4, 0, 3, 1, 5, 6, 2)
    return np.ascontiguousarray(w.reshape(P, E, CI2, CO2, QW)).astype(BF16_NP)


def _prep_rwt(rW):
    # [interm, cin] -> transpose -> [cin_in(128), cin_chunk, interm]
    t = np.asarray(rW, np.float32).T.reshape(CI2, P, C).transpose(1, 0, 2)
    return np.ascontiguousarray(t).astype(BF16_NP)


def _prep_vec(v):
    return np.ascontiguousarray(np.asarray(v, np.float32).reshape(CI2, P).T)


def _fold_bn(g, b, m, v):
    inv = np.asarray(g, np.float32) / np.sqrt(np.asarray(v, np.float32) + EPS)
    shift = np.asarray(b, np.float32) - np.asarray(m, np.float32) * inv
    return _prep_vec(inv), _prep_vec(shift)


def _mask4():
    m = np.zeros((P, E), np.float32)
    for e in range(E):
        lo = 64 * (e % 2)
        m[lo : lo + 64, e] = 1.0 / 64.0
    return m


def _prep_inputs(inputs):
    inv1, shift1 = _fold_bn(inputs["bn1_gamma"], inputs["bn1_beta"],
                            inputs["bn1_mean"], inputs["bn1_var"])
    inv2, shift2 = _fold_bn(inputs["bn2_gamma"], inputs["bn2_beta"],
                            inputs["bn2_mean"], inputs["bn2_var"])
    fblob = np.concatenate(
        [_prep_vec(inputs["r1_b"]), _prep_vec(inputs["r2_b"]),
         inv1, shift1, inv2, shift2, _mask4()], axis=1
    )
    rwt = np.stack([_prep_rwt(inputs["r1_W"]), _prep_rwt(inputs["r2_W"])], axis=1)
    shared = {
        "ew1": _prep_ew(inputs["e1_w"]),
        "ew2": _prep_ew(inputs["e2_w"]),
        "rwt": np.ascontiguousarray(rwt),
        "fblob": np.ascontiguousarray(fblob),
    }
    # pre-padded bf16 x: [core, B_LOC, CI2, P, 34, 34] with zero borders
    x = np.asarray(inputs["x"], np.float32).reshape(N_CORES, B_LOC, CI2, P, 32, 32)
    xp = np.zeros((N_CORES, B_LOC, CI2, P, PADW, PADW), BF16_NP)
    xp[..., 1:33, 1:33] = x.astype(BF16_NP)
    xp = xp.reshape(N_CORES, B_LOC, CI2, P, PADHW)
    return shared, xp


def _run(inputs, trace=False):
    from concourse.bass_utils import run_bass_kernel_spmd

    nc = _build_nc()
    shared, xp = _prep_inputs(inputs)
    in_maps = [{"xp": xp[c], **shared} for c in range(N_CORES)]
    r = run_bass_kernel_spmd(nc, in_maps, list(range(N_CORES)), trace=trace)
    out = np.stack([np.asarray(r.results[c]["out"]) for c in range(N_CORES)])
    return out.reshape(32, C, 32, 32).astype(np.float32), r


def kernel(**inputs):
    out, _ = _run(inputs, trace=False)
    return out


def _install_ntff_shim():
    """The image's antenv package lacks axon_hooks; recreate it and register
    the ctypes NTFF profile hook the way trn_boot would have."""
    import sys
    import types

    if "antenv.axon_hooks" in sys.modules:
        return
    mod = types.ModuleType("antenv.axon_hooks")
    state = {"hook": None}
    mod.set_axon_ntff_profile_hook = lambda h: state.update(hook=h)
    mod.get_axon_ntff_profile_hook = lambda: state["hook"]
    sys.modules["antenv.axon_hooks"] = mod
    import antenv

    antenv.axon_hooks = mod
    try:
        from trn_agent_boot.trn_boot import _ntff_profile_via_ctypes

        mod.set_axon_ntff_profile_hook(
            _ntff_profile_via_ctypes("/opt/axon/libaxon_pjrt.so")
        )
    except Exception as e:  # degrade to no tracing
        print(f"ntff shim failed: {e}")


def run_traced(inputs):
    _install_ntff_shim()
    out, r = _run(inputs, trace=True)
    return out, r


def run_sim(inputs):
    """CoreSim of core 0's shard. Returns [B_LOC, C, 32, 32]."""
    from concourse.bass_interp import CoreSim

    nc = _build_nc()
    shared, xp = _prep_inputs(inputs)
    sim = CoreSim(nc)
    for k, v in {"xp": xp[0], **shared}.items():
        sim.tensor(k)[:] = v
    sim.simulate()
    return np.asarray(sim.tensor("out")).reshape(B_LOC, C, 32, 32).copy()
